# revision 21
# baseline (speedup 1.0000x reference)
"""ChebConv (K=2) + temporal Conv1d GNN kernel for 8 Trainium2 NeuronCores.

Strategy (data-parallel over destination nodes):
  - Node axis padded to 50176 = 392 blocks of 128. Blocks are grouped into
    49 slot-groups of 8 (one block per core per slot), matched by per-half
    edge counts (local search) so the shared static program's padded chunk
    counts stay close to each core's real counts.
  - Host precomputes w_hat (edge weights of -D^-1/2 A D^-1/2), sorts each
    (core, slot, src-half) edge group by dst subblock, and pads to a
    multiple of 128 (the padded count = max over the 8 cores).
  - Per slot the device SWDGE-gathers the edges' source rows from an fp8e3
    copy of x padded to 512-B rows (one descriptor per edge at the DMA
    cost model's 512-B sweet spot), builds a weighted one-hot [128, J, 128]
    on DVE, and segment-sums via TensorE with the gathered rows as lhsT so
    each 128-edge chunk costs only span*32 moving rows; the result lands
    feature-major (transposed) which is exactly what the combine needs.
  - Chebyshev combine + temporal conv collapse into 6 dense fp16 matmuls
    with host-prefolded [128, 384] weights + a K=1 bias matmul; LeakyReLU
    runs on the Activation engine; output written fp16 and reassembled on
    host.
"""

import numpy as np
import ml_dtypes

N = 50000
E = 1600000
W = 12
C = 32
WC = W * C            # 384
NCORES = 8
P = 128
NPAD = 50176          # 392 * 128
NB = NPAD // P        # 392
SLOTS = NB // NCORES  # 49
HALF = NPAD // 2      # 25088
ROWE = 512            # gathered row elements (fp8), 384 data + 128 pad
WH_SCALE = 16.0       # fold 1/16 into path-1 combine mats

_cache = {}


def _assign_blocks(cnt):
    """Partition 392 blocks into 49 groups of 8, minimizing
    sum_i sum_h max_c ceil(cnt[g, h]/128)."""
    order = np.argsort(cnt[:, 0], kind="stable")
    groups = order.reshape(SLOTS, NCORES).copy()

    def group_cost(g):
        ch = -(-cnt[g, :] // P)      # [8, 2] ceil
        return int(ch.max(axis=0).sum())

    costs = np.array([group_cost(groups[i]) for i in range(SLOTS)])
    rng = np.random.default_rng(0)
    for _ in range(30000):
        i1, i2 = rng.integers(0, SLOTS, 2)
        if i1 == i2:
            continue
        c1, c2 = rng.integers(0, NCORES, 2)
        g1, g2 = groups[i1].copy(), groups[i2].copy()
        g1[c1], g2[c2] = g2[c2], g1[c1]
        n1, n2 = group_cost(g1), group_cost(g2)
        if n1 + n2 < costs[i1] + costs[i2]:
            groups[i1], groups[i2] = g1, g2
            costs[i1], costs[i2] = n1, n2
    return groups


def _host_prep(x, A, Ew):
    src = np.asarray(A[0], np.int64)
    dst = np.asarray(A[1], np.int64)
    Ew = np.asarray(Ew, np.float32)

    deg = np.bincount(dst, weights=Ew.astype(np.float64), minlength=N).astype(np.float32)
    dinv = np.where(deg > 0, 1.0 / np.sqrt(np.maximum(deg, 1e-12)), 0.0).astype(np.float32)
    w_hat = (-dinv[src] * Ew * dinv[dst]).astype(np.float32)

    # node-major x: fp8e3 gather rows padded to 512 B; fp16 transposed copy
    xr = np.asarray(x, np.float32).transpose(1, 0, 2).reshape(N, WC)
    xrow8 = np.zeros((NPAD, ROWE), ml_dtypes.float8_e3m4)
    xrow8[:N, :WC] = xr.astype(ml_dtypes.float8_e3m4)
    xrow16 = np.zeros((NPAD, WC), np.float16)
    xrow16[:N] = xr.astype(np.float16)

    blk = dst >> 7
    hh = (src >= HALF).astype(np.int64)
    sb = (dst >> 5) & 3
    cnt_bh = np.bincount(blk * 2 + hh, minlength=NB * 2).reshape(NB, 2)

    groups = _assign_blocks(cnt_bh)          # [SLOTS, 8] block ids
    slot_of = np.zeros(NB, np.int64)
    core_of = np.zeros(NB, np.int64)
    for i in range(SLOTS):
        for c in range(NCORES):
            slot_of[groups[i, c]] = i
            core_of[groups[i, c]] = c

    # static chunk counts
    Jh = np.zeros((SLOTS, 2), np.int64)
    for i in range(SLOTS):
        ch = -(-cnt_bh[groups[i]] // P)      # [8, 2]
        Jh[i] = np.maximum(1, ch.max(axis=0))
    Ji = Jh.sum(axis=1)
    JT = int(Ji.sum())
    joff = np.zeros(SLOTS + 1, np.int64)
    np.cumsum(Ji, out=joff[1:])
    IWT = JT * 8

    # sort edges once by (core, slot, h, s)
    gid = ((core_of[blk] * SLOTS + slot_of[blk]) * 2 + hh) * 4 + sb
    order = np.argsort(gid, kind="stable")
    src_s = src[order]
    dstl_s = (dst[order] & 31).astype(np.float16)
    what_s = (w_hat[order] * WH_SCALE).astype(np.float16)
    sb_s = sb[order]
    counts4 = np.bincount(gid, minlength=NB * 8)
    gstart = np.zeros(NB * 8 + 1, np.int64)
    np.cumsum(counts4, out=gstart[1:])

    idx16 = np.zeros((NCORES, 128, IWT), np.int16)
    xT = np.zeros((NCORES, SLOTS * P, WC), np.float16)
    out_blocks = groups                       # for reassembly

    # per-(core, chunk) edge payloads; plane split happens after spans known
    D_all = np.zeros((NCORES, JT, P), np.float16)   # dst & 31
    W_all = np.zeros((NCORES, JT, P), np.float16)   # w_hat * 16
    S_all = np.zeros((NCORES, JT, P), np.int8)      # subblock 0..3
    S_all -= 1                                      # padding marker

    span_lo = np.full((SLOTS, int(Ji.max())), 4, np.int64)
    span_hi = np.full((SLOTS, int(Ji.max())), -1, np.int64)

    for i in range(SLOTS):
        J0 = int(Jh[i, 0])
        for c in range(NCORES):
            b = groups[i, c]
            # transposed x for this block: xT[i*128+p, t*128+nn] = x[node nn, feat t*128+p]
            xb = xrow16[b * P:(b + 1) * P, :]              # [128 nodes, 384]
            xT[c, i * P:(i + 1) * P, :] = \
                xb.T.reshape(3, P, P).transpose(1, 0, 2).reshape(P, WC)
            for h in range(2):
                Jg = int(Jh[i, h])
                L = Jg * P
                V = np.zeros(L, np.int16)
                g0 = ((c * SLOTS + i) * 2 + h) * 4
                n = int(gstart[g0 + 4] - gstart[g0])
                sl = slice(int(gstart[g0]), int(gstart[g0] + n))
                V[:n] = (src_s[sl] - h * HALF).astype(np.int16)
                co = int(joff[i] + (J0 if h else 0))
                D_all[c, co:co + Jg].reshape(-1)[:n] = dstl_s[sl]
                W_all[c, co:co + Jg].reshape(-1)[:n] = what_s[sl]
                S_all[c, co:co + Jg].reshape(-1)[:n] = sb_s[sl]
                svals = sb_s[sl]
                for jj in range(Jg):
                    a0, a1 = jj * P, min(jj * P + P, n)
                    if a0 < a1:
                        gj = co - int(joff[i]) + jj
                        span_lo[i, gj] = min(span_lo[i, gj], int(svals[a0]))
                        span_hi[i, gj] = max(span_hi[i, gj], int(svals[a1 - 1]))
                idx_blk = V.reshape(-1, 16).T               # [16, L/16]
                idx16[c, :, co * 8: co * 8 + L // 16] = np.tile(idx_blk, (8, 1))

    # static plane structure: chunk j uses planes q=0..span-1, sigma = lo_j + q
    NPL = 2
    for i in range(SLOTS):
        for jj in range(int(Ji[i])):
            lo, hi = int(span_lo[i, jj]), int(span_hi[i, jj])
            if hi >= lo:
                NPL = max(NPL, hi - lo + 1)

    # dmwh: per-slot contiguous [dstl | whplane0 | whplane1 ...] blocks
    dmwh = np.zeros((NCORES, 128, JT * (1 + NPL)), np.float16)
    for i in range(SLOTS):
        jo, J = int(joff[i]), int(Ji[i])
        base = jo * (1 + NPL)
        for c in range(NCORES):
            dmwh[c, :, base:base + J] = D_all[c, jo:jo + J].T
            for q in range(NPL):
                # weight plane q: only edges with s == lo_j + q
                Wq = W_all[c, jo:jo + J].copy()
                for jj in range(J):
                    lo = int(span_lo[i, jj])
                    if lo > 3:
                        lo = 0
                    mask = S_all[c, jo + jj] != (lo + q)
                    Wq[jj][mask] = 0
                dmwh[c, :, base + (1 + q) * J:base + (2 + q) * J] = Wq.T

    # static matmul plan per slot: [(jj, q, sigma)] ; sigma = lo_j + q
    plans = []
    for i in range(SLOTS):
        plan = []
        present = set()
        for jj in range(int(Ji[i])):
            lo, hi = int(span_lo[i, jj]), int(span_hi[i, jj])
            if hi < lo:
                lo, hi = 0, 0
            for q in range(hi - lo + 1):
                plan.append((jj, q, lo + q))
                present.add(lo + q)
        assert present == {0, 1, 2, 3}, (i, present)
        plans.append(plan)

    return (xrow8, xT, idx16, dmwh, NPL, Jh, Ji, joff, JT, IWT,
            tuple(tuple(p) for p in map(tuple, plans)), out_blocks)


def _fold_weights(Wcheb, bcheb, Wconv, bconv):
    Wcheb = np.asarray(Wcheb, np.float32)
    bcheb = np.asarray(bcheb, np.float32)
    Wconv = np.asarray(Wconv, np.float32)
    bconv = np.asarray(bconv, np.float32)
    # mats[path, gi]: [128 featin, 384 featout]
    mats = np.zeros((2, 3, P, WC), np.float32)
    for path in range(2):
        for gi in range(3):
            for wl in range(4):
                wi = 4 * gi + wl
                for k in range(3):
                    wo = wi - k + 1
                    if not (0 <= wo < W):
                        continue
                    Cm = Wcheb[wi, path] @ Wconv[:, :, k].T      # [ci, co]
                    mats[path, gi, 32 * wl:32 * wl + 32, 32 * wo:32 * wo + 32] = Cm
    mats[1] /= WH_SCALE
    mats_sb = np.ascontiguousarray(
        mats.reshape(6, P, WC).transpose(1, 0, 2).reshape(P, 6 * WC)).astype(np.float16)
    bias = np.zeros((W, C), np.float32)
    for wo in range(W):
        bias[wo] = bconv.copy()
        for k in range(3):
            wi = wo + k - 1
            if 0 <= wi < W:
                bias[wo] += bcheb[wi] @ Wconv[:, :, k].T
    bias_sb = bias.reshape(1, WC).astype(np.float16)
    return mats_sb, bias_sb


def _build_program(Jh, Ji, joff, JT, IWT, NPL, plans):
    import concourse.bacc as bacc
    import concourse.tile as tile
    from concourse import mybir
    import concourse.bass as bass  # noqa

    nc = bacc.Bacc("TRN2", target_bir_lowering=False, debug=False,
                   num_devices=NCORES)
    f16, f32, i16 = mybir.dt.float16, mybir.dt.float32, mybir.dt.int16
    f8 = mybir.dt.float8e3
    xrow8 = nc.dram_tensor("xrow8", [NPAD, ROWE], f8, kind="ExternalInput")
    xTd = nc.dram_tensor("xTd", [SLOTS * P, WC], f16, kind="ExternalInput")
    idx16 = nc.dram_tensor("idx16", [128, IWT], i16, kind="ExternalInput")
    dmwh = nc.dram_tensor("dmwh", [128, JT * (1 + NPL)], f16, kind="ExternalInput")
    mats = nc.dram_tensor("mats", [128, 6 * WC], f16, kind="ExternalInput")
    biasd = nc.dram_tensor("biasd", [1, WC], f16, kind="ExternalInput")
    onesd = nc.dram_tensor("onesd", [1, 128], f16, kind="ExternalInput")
    zerod = nc.dram_tensor("zerod", [1, WC], f16, kind="ExternalInput")
    iota = nc.dram_tensor("iota", [128, 32], f16, kind="ExternalInput")
    out_pc = nc.dram_tensor("out_pc", [SLOTS * P, WC], f16, kind="ExternalOutput")

    JMAX = int(Ji.max())

    with tile.TileContext(nc) as tc:
        with tc.tile_pool(name="const", bufs=1) as cp, \
             tc.tile_pool(name="sb", bufs=3) as sbp, \
             tc.tile_pool(name="xgp", bufs=3) as xgp, \
             tc.tile_pool(name="pst", bufs=4, space="PSUM") as pst, \
             tc.tile_pool(name="psy", bufs=4, space="PSUM") as psy:
            mats_t = cp.tile([128, 6 * WC], f16)
            nc.sync.dma_start(out=mats_t[:], in_=mats.ap())
            bias_t = cp.tile([1, WC], f16)
            nc.sync.dma_start(out=bias_t[:], in_=biasd.ap())
            ones_t = cp.tile([1, 128], f16)
            nc.sync.dma_start(out=ones_t[:], in_=onesd.ap())
            zero_t = cp.tile([1, WC], f16)
            nc.sync.dma_start(out=zero_t[:], in_=zerod.ap())
            iota_t = cp.tile([128, 32], f16)
            nc.sync.dma_start(out=iota_t[:], in_=iota.ap())

            for i in range(SLOTS):
                J0, J1 = int(Jh[i, 0]), int(Jh[i, 1])
                J = J0 + J1
                jo = int(joff[i])
                plan = plans[i]

                idx_t = sbp.tile([128, JMAX * 8], i16, tag="idx")
                nc.sync.dma_start(out=idx_t[:, :J * 8],
                                  in_=idx16.ap()[:, jo * 8:(jo + J) * 8])
                dw_t = sbp.tile([128, JMAX * (1 + NPL)], f16, tag="dw")
                base = jo * (1 + NPL)
                nc.sync.dma_start(out=dw_t[:, :J * (1 + NPL)],
                                  in_=dmwh.ap()[:, base:base + J * (1 + NPL)])

                xg = xgp.tile([128, JMAX, ROWE], f8, tag="xg")
                nc.gpsimd.dma_gather(
                    xg[:, 0:J0, :], xrow8.ap()[0:HALF, :],
                    idx_t[:, 0:J0 * 8], J0 * 128, J0 * 128, ROWE,
                    single_packet=False)
                nc.gpsimd.dma_gather(
                    xg[:, J0:J, :], xrow8.ap()[HALF:NPAD, :],
                    idx_t[:, J0 * 8:J * 8], J1 * 128, J1 * 128, ROWE,
                    single_packet=False)

                eq = sbp.tile([128, JMAX, 32], f16, tag="eq")
                nc.vector.tensor_tensor(
                    out=eq[:, :J, :],
                    in0=dw_t[:, :J].unsqueeze(2).to_broadcast([128, J, 32]),
                    in1=iota_t[:].unsqueeze(1).to_broadcast([128, J, 32]),
                    op=mybir.AluOpType.is_equal)
                wm = sbp.tile([128, NPL * JMAX, 32], f8, tag="wm")
                for q in range(NPL):
                    nc.vector.tensor_tensor(
                        out=wm[:, q * JMAX:q * JMAX + J, :],
                        in0=eq[:, :J, :],
                        in1=dw_t[:, (1 + q) * J:(2 + q) * J]
                            .unsqueeze(2).to_broadcast([128, J, 32]),
                        op=mybir.AluOpType.mult)

                # flipped segment-sum: t1[t*128+d-col] = [128 feat, dst]
                # PSUM start=True wipes the written partitions' whole 2KB
                # bank, so the slot uses ONE accumulation group per bank:
                # first matmul starts, everything else accumulates.
                t1p = pst.tile([128, 512], f32, space="PSUM", tag="t1")
                # zeroing matmul: start=True wipes the bank and the full-region
                # write creates a WAW dep that orders it before the accumulates
                nc.tensor.matmul(out=t1p[:, 0:WC], lhsT=ones_t[:], rhs=zero_t[:],
                                 start=True, stop=True, skip_group_check=True,
                                 tile_position=(0, 0))
                by_chunk = {}
                for (jj, q, s) in plan:
                    by_chunk.setdefault(jj, []).append((q, s))
                for jj in sorted(by_chunk):
                    for t in range(3):
                        for (q, s) in by_chunk[jj]:
                            nc.tensor.matmul(
                                out=t1p[:, 128 * t + 32 * s:128 * t + 32 * s + 32],
                                lhsT=xg[:, jj:jj + 1, 128 * t:128 * t + 128],
                                rhs=wm[:, q * JMAX + jj:q * JMAX + jj + 1, :],
                                start=False, stop=False, skip_group_check=True,
                                tile_position=(0, 0))
                t1s = sbp.tile([128, WC], f16, tag="t1s")
                nc.scalar.copy(out=t1s[:], in_=t1p[:, 0:WC])

                xt = sbp.tile([128, WC], f16, tag="xt")
                nc.sync.dma_start(out=xt[:], in_=xTd.ap()[i * P:(i + 1) * P, :])

                pyt = psy.tile([128, 512], f32, space="PSUM", tag="y")
                nc.tensor.matmul(out=pyt[:, 0:WC], lhsT=ones_t[:], rhs=bias_t[:],
                                 start=True, stop=False, tile_position=(0, 0))
                for path in range(2):
                    srct = xt if path == 0 else t1s
                    for gi in range(3):
                        pi = path * 3 + gi
                        nc.tensor.matmul(
                            out=pyt[:, 0:WC],
                            lhsT=srct[:, 128 * gi:128 * gi + 128],
                            rhs=mats_t[:, pi * WC:(pi + 1) * WC],
                            start=False, stop=(pi == 5),
                            tile_position=(0, 0))

                osb = sbp.tile([128, WC], f16, tag="osb")
                nc.scalar.activation(out=osb[:], in_=pyt[:, 0:WC],
                                     func=mybir.ActivationFunctionType.Lrelu,
                                     bias=0.0, scale=1.0, alpha=0.01)
                # issue from ACT queue: keeps SP's in-order stream pure
                # prefetch (out-DMA waits on compute and would block it)
                nc.scalar.dma_start(out=out_pc.ap()[i * P:(i + 1) * P, :], in_=osb[:])

    nc.compile()
    return nc


def kernel(x, A, Ew, Wcheb, bcheb, Wconv, bconv, batch_size=1):
    from concourse.bass_utils import run_bass_kernel_spmd

    (xrow8, xT, idx16, dmwh, NPL, Jh, Ji, joff, JT, IWT, plans,
     out_blocks) = _host_prep(x, A, Ew)
    mats_sb, bias_sb = _fold_weights(Wcheb, bcheb, Wconv, bconv)

    key = (JT, IWT, NPL, plans)
    if key not in _cache:
        _cache[key] = _build_program(Jh, Ji, joff, JT, IWT, NPL, plans)
    nc = _cache[key]

    iota_np = np.tile(np.arange(32, dtype=np.float16)[None, :], (128, 1))
    ones_np = np.ones((1, 128), np.float16)
    zero_np = np.zeros((1, WC), np.float16)
    in_maps = []
    for c in range(NCORES):
        in_maps.append(dict(
            xrow8=xrow8, xTd=xT[c], idx16=idx16[c],
            dmwh=dmwh[c], mats=mats_sb, biasd=bias_sb,
            onesd=ones_np, zerod=zero_np, iota=iota_np))
    res = run_bass_kernel_spmd(nc, in_maps, core_ids=list(range(NCORES)))

    out_full = np.zeros((NPAD, WC), np.float32)
    for c in range(NCORES):
        o = np.asarray(res.results[c]["out_pc"], np.float32)
        for i in range(SLOTS):
            b = out_blocks[i, c]
            out_full[b * P:(b + 1) * P] = o[i * P:(i + 1) * P]
    return np.ascontiguousarray(out_full[:N]).reshape(N, W, C)


# revision 32
# speedup vs baseline: 1.4274x; 1.4274x over previous
"""ChebConv (K=2) + temporal Conv1d GNN kernel for 8 Trainium2 NeuronCores.

Strategy (data-parallel over destination nodes):
  - Node axis padded to 50176 = 392 blocks of 128. Blocks are grouped into
    49 slot-groups of 8 (one block per core per slot), matched by per-half
    edge counts (local search) so the shared static program's padded chunk
    counts stay close to each core's real counts.
  - Host precomputes w_hat (edge weights of -D^-1/2 A D^-1/2), sorts each
    (core, slot, src-half) edge group by dst subblock, and pads to a
    multiple of 128 (the padded count = max over the 8 cores).
  - Per slot the device SWDGE-gathers the edges' source rows from an fp8e3
    copy of x padded to 512-B rows (one descriptor per edge at the DMA
    cost model's 512-B sweet spot), builds a weighted one-hot [128, J, 128]
    on DVE, and segment-sums via TensorE with the gathered rows as lhsT so
    each 128-edge chunk costs only span*32 moving rows; the result lands
    feature-major (transposed) which is exactly what the combine needs.
  - Chebyshev combine + temporal conv collapse into 6 dense fp16 matmuls
    with host-prefolded [128, 384] weights + a K=1 bias matmul; LeakyReLU
    runs on the Activation engine; output written fp16 and reassembled on
    host.
"""

import numpy as np
import ml_dtypes

N = 50000
E = 1600000
W = 12
C = 32
WC = W * C            # 384
NCORES = 8
P = 128
NPAD = 50176          # 392 * 128
NB = NPAD // P        # 392
SLOTS = NB // NCORES  # 49
HALF = NPAD // 2      # 25088
ROWE = 512            # gathered row elements (fp8), 384 data + 128 pad
WH_SCALE = 16.0       # fold 1/16 into path-1 combine mats

_cache = {}


def _assign_blocks(cnt):
    """Partition 392 blocks into 49 groups of 8, minimizing
    sum_i sum_h max_c ceil(cnt[g, h]/128)."""
    order = np.argsort(cnt[:, 0], kind="stable")
    groups = order.reshape(SLOTS, NCORES).copy()

    def group_cost(g):
        ch = -(-cnt[g, :] // P)      # [8, 2] ceil
        return int(ch.max(axis=0).sum())

    costs = np.array([group_cost(groups[i]) for i in range(SLOTS)])
    rng = np.random.default_rng(0)
    for _ in range(30000):
        i1, i2 = rng.integers(0, SLOTS, 2)
        if i1 == i2:
            continue
        c1, c2 = rng.integers(0, NCORES, 2)
        g1, g2 = groups[i1].copy(), groups[i2].copy()
        g1[c1], g2[c2] = g2[c2], g1[c1]
        n1, n2 = group_cost(g1), group_cost(g2)
        if n1 + n2 < costs[i1] + costs[i2]:
            groups[i1], groups[i2] = g1, g2
            costs[i1], costs[i2] = n1, n2
    return groups


def _host_prep(x, A, Ew):
    src = np.asarray(A[0], np.int64)
    dst = np.asarray(A[1], np.int64)
    Ew = np.asarray(Ew, np.float32)

    deg = np.bincount(dst, weights=Ew.astype(np.float64), minlength=N).astype(np.float32)
    dinv = np.where(deg > 0, 1.0 / np.sqrt(np.maximum(deg, 1e-12)), 0.0).astype(np.float32)
    w_hat = (-dinv[src] * Ew * dinv[dst]).astype(np.float32)

    # node-major x: fp8e3 gather rows padded to 512 B; fp16 transposed copy
    xr = np.asarray(x, np.float32).transpose(1, 0, 2).reshape(N, WC)
    xrow8 = np.zeros((NPAD, ROWE), ml_dtypes.float8_e3m4)
    xrow8[:N, :WC] = xr.astype(ml_dtypes.float8_e3m4)
    xrow16 = np.zeros((NPAD, WC), np.float16)
    xrow16[:N] = xr.astype(np.float16)

    blk = dst >> 7
    hh = (src >= HALF).astype(np.int64)
    sb = (dst >> 5) & 3
    cnt_bh = np.bincount(blk * 2 + hh, minlength=NB * 2).reshape(NB, 2)

    groups = _assign_blocks(cnt_bh)          # [SLOTS, 8] block ids
    slot_of = np.zeros(NB, np.int64)
    core_of = np.zeros(NB, np.int64)
    for i in range(SLOTS):
        for c in range(NCORES):
            slot_of[groups[i, c]] = i
            core_of[groups[i, c]] = c

    # static chunk counts
    Jh = np.zeros((SLOTS, 2), np.int64)
    for i in range(SLOTS):
        ch = -(-cnt_bh[groups[i]] // P)      # [8, 2]
        Jh[i] = np.maximum(1, ch.max(axis=0))
    Ji = Jh.sum(axis=1)
    JT = int(Ji.sum())
    joff = np.zeros(SLOTS + 1, np.int64)
    np.cumsum(Ji, out=joff[1:])
    IWT = JT * 8

    # sort edges once by (core, slot, h, s)
    gid = ((core_of[blk] * SLOTS + slot_of[blk]) * 2 + hh) * 4 + sb
    order = np.argsort(gid, kind="stable")
    src_s = src[order]
    dstl_s = (dst[order] & 31).astype(np.float16)
    what_s = (w_hat[order] * WH_SCALE).astype(np.float16)
    sb_s = sb[order]
    counts4 = np.bincount(gid, minlength=NB * 8)
    gstart = np.zeros(NB * 8 + 1, np.int64)
    np.cumsum(counts4, out=gstart[1:])

    idx16 = np.zeros((NCORES, 128, IWT), np.int16)
    xT = np.zeros((NCORES, SLOTS * P, WC), np.float16)
    out_blocks = groups                       # for reassembly
    # (idx16 is later packed together with dstl/planes into `comb`)

    # per-(core, chunk) edge payloads; plane split happens after spans known
    D_all = np.zeros((NCORES, JT, P), np.float16)   # dst & 31
    W_all = np.zeros((NCORES, JT, P), np.float16)   # w_hat * 16
    S_all = np.zeros((NCORES, JT, P), np.int8)      # subblock 0..3
    S_all -= 1                                      # padding marker

    span_lo = np.full((SLOTS, int(Ji.max())), 4, np.int64)
    span_hi = np.full((SLOTS, int(Ji.max())), -1, np.int64)

    for i in range(SLOTS):
        J0 = int(Jh[i, 0])
        for c in range(NCORES):
            b = groups[i, c]
            # transposed x for this block: xT[i*128+p, t*128+nn] = x[node nn, feat t*128+p]
            xb = xrow16[b * P:(b + 1) * P, :]              # [128 nodes, 384]
            xT[c, i * P:(i + 1) * P, :] = \
                xb.T.reshape(3, P, P).transpose(1, 0, 2).reshape(P, WC)
            for h in range(2):
                Jg = int(Jh[i, h])
                L = Jg * P
                V = np.zeros(L, np.int16)
                g0 = ((c * SLOTS + i) * 2 + h) * 4
                n = int(gstart[g0 + 4] - gstart[g0])
                sl = slice(int(gstart[g0]), int(gstart[g0] + n))
                V[:n] = (src_s[sl] - h * HALF).astype(np.int16)
                co = int(joff[i] + (J0 if h else 0))
                D_all[c, co:co + Jg].reshape(-1)[:n] = dstl_s[sl]
                W_all[c, co:co + Jg].reshape(-1)[:n] = what_s[sl]
                S_all[c, co:co + Jg].reshape(-1)[:n] = sb_s[sl]
                svals = sb_s[sl]
                for jj in range(Jg):
                    a0, a1 = jj * P, min(jj * P + P, n)
                    if a0 < a1:
                        gj = co - int(joff[i]) + jj
                        span_lo[i, gj] = min(span_lo[i, gj], int(svals[a0]))
                        span_hi[i, gj] = max(span_hi[i, gj], int(svals[a1 - 1]))
                idx_blk = V.reshape(-1, 16).T               # [16, L/16]
                idx16[c, :, co * 8: co * 8 + L // 16] = np.tile(idx_blk, (8, 1))

    # static plane structure: chunk j uses planes q=0..span-1, sigma = lo_j + q
    NPL = 2
    for i in range(SLOTS):
        for jj in range(int(Ji[i])):
            lo, hi = int(span_lo[i, jj]), int(span_hi[i, jj])
            if hi >= lo:
                NPL = max(NPL, hi - lo + 1)

    # comb: per-slot contiguous [idx (J*8, i16) | dstl (J) | planes (NPL*J)]
    CW = 9 + NPL
    comb = np.zeros((NCORES, 128, JT * CW), np.int16)
    for i in range(SLOTS):
        jo, J = int(joff[i]), int(Ji[i])
        base = jo * CW
        for c in range(NCORES):
            comb[c, :, base:base + J * 8] = idx16[c, :, jo * 8:(jo + J) * 8]
            comb[c, :, base + J * 8:base + J * 9] = \
                D_all[c, jo:jo + J].view(np.int16).T
            for q in range(NPL):
                # weight plane q: only edges with s == lo_j + q
                Wq = W_all[c, jo:jo + J].copy()
                for jj in range(J):
                    lo = int(span_lo[i, jj])
                    if lo > 3:
                        lo = 0
                    mask = S_all[c, jo + jj] != (lo + q)
                    Wq[jj][mask] = 0
                comb[c, :, base + J * (9 + q):base + J * (10 + q)] = \
                    Wq.view(np.int16).T

    # static matmul plan per slot: [(jj, q, sigma)] ; sigma = lo_j + q
    plans = []
    for i in range(SLOTS):
        plan = []
        present = set()
        for jj in range(int(Ji[i])):
            lo, hi = int(span_lo[i, jj]), int(span_hi[i, jj])
            if hi < lo:
                lo, hi = 0, 0
            for q in range(hi - lo + 1):
                plan.append((jj, q, lo + q))
                present.add(lo + q)
        assert present == {0, 1, 2, 3}, (i, present)
        plans.append(plan)

    return (xrow8, xT, comb, NPL, Jh, Ji, joff, JT, IWT,
            tuple(tuple(p) for p in map(tuple, plans)), out_blocks)


def _fold_weights(Wcheb, bcheb, Wconv, bconv):
    Wcheb = np.asarray(Wcheb, np.float32)
    bcheb = np.asarray(bcheb, np.float32)
    Wconv = np.asarray(Wconv, np.float32)
    bconv = np.asarray(bconv, np.float32)
    # mats[path, gi]: [128 featin, 384 featout]
    mats = np.zeros((2, 3, P, WC), np.float32)
    for path in range(2):
        for gi in range(3):
            for wl in range(4):
                wi = 4 * gi + wl
                for k in range(3):
                    wo = wi - k + 1
                    if not (0 <= wo < W):
                        continue
                    Cm = Wcheb[wi, path] @ Wconv[:, :, k].T      # [ci, co]
                    mats[path, gi, 32 * wl:32 * wl + 32, 32 * wo:32 * wo + 32] = Cm
    mats[1] /= WH_SCALE
    mats_sb = np.ascontiguousarray(
        mats.reshape(6, P, WC).transpose(1, 0, 2).reshape(P, 6 * WC)).astype(np.float16)
    bias = np.zeros((W, C), np.float32)
    for wo in range(W):
        bias[wo] = bconv.copy()
        for k in range(3):
            wi = wo + k - 1
            if 0 <= wi < W:
                bias[wo] += bcheb[wi] @ Wconv[:, :, k].T
    bias_sb = bias.reshape(1, WC).astype(np.float16)
    return mats_sb, bias_sb


def _build_program(Jh, Ji, joff, JT, IWT, NPL, plans):
    import concourse.bacc as bacc
    import concourse.tile as tile
    from concourse import mybir
    import concourse.bass as bass  # noqa

    nc = bacc.Bacc("TRN2", target_bir_lowering=False, debug=False,
                   num_devices=NCORES)
    f16, f32, i16 = mybir.dt.float16, mybir.dt.float32, mybir.dt.int16
    f8 = mybir.dt.float8e3
    CW = 9 + NPL
    xrow8 = nc.dram_tensor("xrow8", [NPAD, ROWE], f8, kind="ExternalInput")
    xTd = nc.dram_tensor("xTd", [SLOTS * P, WC], f16, kind="ExternalInput")
    combd = nc.dram_tensor("combd", [128, JT * CW], i16, kind="ExternalInput")
    mats = nc.dram_tensor("mats", [128, 6 * WC], f16, kind="ExternalInput")
    biasd = nc.dram_tensor("biasd", [1, WC], f16, kind="ExternalInput")
    onesd = nc.dram_tensor("onesd", [1, 128], f16, kind="ExternalInput")
    zerod = nc.dram_tensor("zerod", [1, WC], f16, kind="ExternalInput")
    iota = nc.dram_tensor("iota", [128, 32], f16, kind="ExternalInput")
    out_pc = nc.dram_tensor("out_pc", [SLOTS * P, WC], f16, kind="ExternalOutput")

    JMAX = int(Ji.max())

    with tile.TileContext(nc) as tc:
        with tc.tile_pool(name="const", bufs=1) as cp, \
             tc.tile_pool(name="sb", bufs=3) as sbp, \
             tc.tile_pool(name="xgp", bufs=3) as xgp, \
             tc.tile_pool(name="pst", bufs=4, space="PSUM") as pst, \
             tc.tile_pool(name="psy", bufs=4, space="PSUM") as psy:
            mats_t = cp.tile([128, 6 * WC], f16)
            nc.sync.dma_start(out=mats_t[:], in_=mats.ap())
            bias_t = cp.tile([1, WC], f16)
            nc.sync.dma_start(out=bias_t[:], in_=biasd.ap())
            ones_t = cp.tile([1, 128], f16)
            nc.sync.dma_start(out=ones_t[:], in_=onesd.ap())
            zero_t = cp.tile([1, WC], f16)
            nc.sync.dma_start(out=zero_t[:], in_=zerod.ap())
            iota_t = cp.tile([128, 32], f16)
            nc.sync.dma_start(out=iota_t[:], in_=iota.ap())

            def issue_copies(i):
                J = int(Ji[i])
                coff = int(joff[i]) * CW
                comb_t = sbp.tile([128, JMAX * CW], i16, tag="comb",
                                  name=f"comb{i}")
                nc.sync.dma_start(out=comb_t[:, :J * CW],
                                  in_=combd.ap()[:, coff:coff + J * CW])
                xt = sbp.tile([128, WC], f16, tag="xt", name=f"xt{i}")
                nc.sync.dma_start(out=xt[:], in_=xTd.ap()[i * P:(i + 1) * P, :])
                return comb_t, xt

            # prefetch distance 2: input copies land on the DMA queue well
            # before the slot's gathers need them
            pending = [issue_copies(0), issue_copies(1)]
            for i in range(SLOTS):
                J0, J1 = int(Jh[i, 0]), int(Jh[i, 1])
                J = J0 + J1
                plan = plans[i]
                comb_t, xt = pending.pop(0)
                if i + 2 < SLOTS:
                    pending.append(issue_copies(i + 2))

                xg = xgp.tile([128, JMAX, ROWE], f8, tag="xg")
                nc.gpsimd.dma_gather(
                    xg[:, 0:J0, :], xrow8.ap()[0:HALF, :],
                    comb_t[:, 0:J0 * 8], J0 * 128, J0 * 128, ROWE,
                    single_packet=False)
                nc.gpsimd.dma_gather(
                    xg[:, J0:J, :], xrow8.ap()[HALF:NPAD, :],
                    comb_t[:, J0 * 8:J * 8], J1 * 128, J1 * 128, ROWE,
                    single_packet=False)

                eq = sbp.tile([128, JMAX, 32], f16, tag="eq")
                nc.vector.tensor_tensor(
                    out=eq[:, :J, :],
                    in0=comb_t[:, J * 8:J * 9].bitcast(f16)
                        .unsqueeze(2).to_broadcast([128, J, 32]),
                    in1=iota_t[:].unsqueeze(1).to_broadcast([128, J, 32]),
                    op=mybir.AluOpType.is_equal)
                wm = sbp.tile([128, NPL * JMAX, 32], f8, tag="wm")
                for q in range(NPL):
                    nc.vector.tensor_tensor(
                        out=wm[:, q * JMAX:q * JMAX + J, :],
                        in0=eq[:, :J, :],
                        in1=comb_t[:, J * (9 + q):J * (10 + q)].bitcast(f16)
                            .unsqueeze(2).to_broadcast([128, J, 32]),
                        op=mybir.AluOpType.mult)

                # flipped segment-sum: t1[t*128+d-col] = [128 feat, dst]
                # PSUM start=True wipes the written partitions' whole 2KB
                # bank, so the slot uses ONE accumulation group per bank:
                # first matmul starts, everything else accumulates.
                t1p = pst.tile([128, 512], f32, space="PSUM", tag="t1")
                # zeroing matmul: start=True wipes the bank and the full-region
                # write creates a WAW dep that orders it before the accumulates
                nc.tensor.matmul(out=t1p[:, 0:WC], lhsT=ones_t[:], rhs=zero_t[:],
                                 start=True, stop=True, skip_group_check=True,
                                 tile_position=(0, 0))
                by_chunk = {}
                for (jj, q, s) in plan:
                    by_chunk.setdefault(jj, []).append((q, s))
                for jj in sorted(by_chunk):
                    for t in range(3):
                        for (q, s) in by_chunk[jj]:
                            nc.tensor.matmul(
                                out=t1p[:, 128 * t + 32 * s:128 * t + 32 * s + 32],
                                lhsT=xg[:, jj:jj + 1, 128 * t:128 * t + 128],
                                rhs=wm[:, q * JMAX + jj:q * JMAX + jj + 1, :],
                                start=False, stop=False, skip_group_check=True,
                                tile_position=(0, 0))
                t1s = sbp.tile([128, WC], f16, tag="t1s")
                nc.scalar.copy(out=t1s[:], in_=t1p[:, 0:WC])

                pyt = psy.tile([128, 512], f32, space="PSUM", tag="y")
                nc.tensor.matmul(out=pyt[:, 0:WC], lhsT=ones_t[:], rhs=bias_t[:],
                                 start=True, stop=False, tile_position=(0, 0))
                for path in range(2):
                    srct = xt if path == 0 else t1s
                    for gi in range(3):
                        pi = path * 3 + gi
                        nc.tensor.matmul(
                            out=pyt[:, 0:WC],
                            lhsT=srct[:, 128 * gi:128 * gi + 128],
                            rhs=mats_t[:, pi * WC:(pi + 1) * WC],
                            start=False, stop=(pi == 5),
                            tile_position=(0, 0))

                osb = sbp.tile([128, WC], f16, tag="osb")
                nc.scalar.activation(out=osb[:], in_=pyt[:, 0:WC],
                                     func=mybir.ActivationFunctionType.Lrelu,
                                     bias=0.0, scale=1.0, alpha=0.01)
                # issue from ACT queue: SP stays pure prefetch (the out-DMA
                # waits on this slot's compute and would stall SP's stream)
                nc.scalar.dma_start(out=out_pc.ap()[i * P:(i + 1) * P, :], in_=osb[:])

    nc.compile()
    return nc


def kernel(x, A, Ew, Wcheb, bcheb, Wconv, bconv, batch_size=1):
    from concourse.bass_utils import run_bass_kernel_spmd

    (xrow8, xT, comb, NPL, Jh, Ji, joff, JT, IWT, plans,
     out_blocks) = _host_prep(x, A, Ew)
    mats_sb, bias_sb = _fold_weights(Wcheb, bcheb, Wconv, bconv)

    key = (JT, IWT, NPL, plans)
    if key not in _cache:
        _cache[key] = _build_program(Jh, Ji, joff, JT, IWT, NPL, plans)
    nc = _cache[key]

    iota_np = np.tile(np.arange(32, dtype=np.float16)[None, :], (128, 1))
    ones_np = np.ones((1, 128), np.float16)
    zero_np = np.zeros((1, WC), np.float16)
    in_maps = []
    for c in range(NCORES):
        in_maps.append(dict(
            xrow8=xrow8, xTd=xT[c], combd=comb[c],
            mats=mats_sb, biasd=bias_sb,
            onesd=ones_np, zerod=zero_np, iota=iota_np))
    res = run_bass_kernel_spmd(nc, in_maps, core_ids=list(range(NCORES)))

    out_full = np.zeros((NPAD, WC), np.float32)
    for c in range(NCORES):
        o = np.asarray(res.results[c]["out_pc"], np.float32)
        for i in range(SLOTS):
            b = out_blocks[i, c]
            out_full[b * P:(b + 1) * P] = o[i * P:(i + 1) * P]
    return np.ascontiguousarray(out_full[:N]).reshape(N, W, C)
